# revision 44
# baseline (speedup 1.0000x reference)
"""Trainium2 Bass kernel for nn_AMMaskedLinear.

Math: the reference's per-sample weight mask is separable:
    weight_mask[b,o,i] = pl[b,i] * ph[b,o] * S[o,i]
with
    present[b,v] = any_j(hidden_rank[b,j] == v)            (v in 0..32)
    pl[b,i] = present[b, r_low[i]]  & (r_low[i]  != 0)
    ph[b,o] = present[b, r_high[o]] & (r_high[o] != 0)
    om[b,o] = present[b, r_high[o]]
    S[o,i]  = (r_low[i] <= r_high[o])
so
    y[b,o]   = ph[b,o] * sum_i (S[o,i]*direction[o,i]) * (pl[b,i]*x[b,i])
    out[b,o] = cscale_b[o] * y[b,o] + om[b,o] * cbias_b[o]
(The Linear(1,out) layers applied to zeros contribute exactly their bias.)

Distribution: OUT (o) is sharded across the 8 NeuronCores; direction is the
only large tensor and each core only touches its own 128-row slice.  All
host-side work is layout/dtype transformation only (transpose / reshape /
broadcast / int->float cast / constant tables); every arithmetic op on the
problem's data runs on the device.

Device pipeline per core (SPMD, raw bass, no collectives):
  1. presence bit-pack: w0 = 1 << min(hr,16), w1 = 1 << relu(hr-15) on a
     [128, 512] layout (partition = (half, b)), OR-tree along free axis,
     OR the two halves -> packed [64, 2] int32.
  2. extract 33 presence bits -> pres [64, 33] bf16, PE-transpose -> [33,64].
  3. one-hot matmuls (bf16, exact on 0/1) gather presence:
     plT[j,b], cbias*om[k,b], (cscale/2)*ph[k,b].
  4. E[j,k] = (Sign(r_high[k]-r_low[j]+0.5)+1) * direction^T[j,k]
     = 2*S*direction — the Sign masks run on the otherwise-idle ScalarE
     in parallel with the DVE pack phase; the x2 is folded into the
     cscale one-hot above.
  5. YT[k,b] = sum_j E[j,k] * (xT[j,b] * plT[j,b])         (PE bf16, f32 acc)
  6. outT[k,b] = (cscale/2*ph)*YT + (cbias*om)             (DVE f32)

Raw bass (not Tile): this neuronxcc build allows only ONE sync-wait per
instruction, which Tile's auto-generated multi-wait drains violate.  Sync
is explicit: one semaphore per DMA, dve/pe/act milestone chains, and
standalone wait_ge instructions.  DVE same-engine RAW edges carry explicit
drain() (the sim race detector requires them; HW needs the pipe flush).
"""

import numpy as np

B, IN, OUT, D = 64, 1024, 1024, 32
NCORES = 8
KSH = OUT // NCORES  # 128 outputs per core
NT = IN // 128       # 8 contraction tiles

# hrp [128, 257] f32 (DMA 1 — smallest, needed first)
H_HR = 0             # [128, 512]  hidden_rank as int16, bitcast (256 cols)
HW_ = 257            # last col unused padding

# aux2 [128, 202] f32 (DMA 3 — small tables)
A_RLPP = 0           # [128, 8]    r_low as f32, partition-major
A_RHBB = 8           # [128, 128]  r_high shard bcast over partitions
A_SHAMT = 136        # [64, 33]    bit-extraction shifts, uint16 (17 f32 cols)
A_IDENT = 153        # [64, 64]    bf16 identity (32 f32 cols), rows 0:64
A_VIOTA = 185        # [33, 1]     permuted value index, f32
A2W = 186

# big128 [128, BW] f32 (DMA 2 — bulk; x and direction shipped as bf16 —
# the device rounds both to bf16 anyway, so this is value-identical)
B_XT = 0             # [128, 8*64]   x^T bf16, tile-major (256 f32 cols)
B_DIRT = 256         # [128, 8*128]  direction^T shard bf16 (512 f32 cols)
B_V33 = 768          # [33, 1408] bf16 vals (704 f32 cols), rows 33.. padding
V_RL = 0             # [33, 1024] r_low bcast          (bf16 units)
V_RH = 1024          # [33, 128]  r_high shard bcast
V_CS = 1152          # [33, 128]  cscale shard bcast
V_CB = 1280          # [33, 128]  cbias shard bcast
VW = 1408            # bf16 units = 704 f32 cols
BW = B_V33 + VW // 2  # 2240

_cached = {}
USE_DRAINS = True  # required: HW does NOT synchronize same-engine RAW without them


def _build_nc():
    import contextlib

    use_drains = USE_DRAINS

    import concourse.bass as bass
    import concourse.mybir as mybir

    f32 = mybir.dt.float32
    bf16 = mybir.dt.bfloat16
    i16 = mybir.dt.int16
    u16 = mybir.dt.uint16
    Alu = mybir.AluOpType
    Act = mybir.ActivationFunctionType

    nc = bass.Bass()

    hrp_h = nc.declare_dram_parameter("hrp", [128, HW_], f32, isOutput=False)
    big_h = nc.declare_dram_parameter("big128", [128, BW], f32, isOutput=False)
    aux2_h = nc.declare_dram_parameter("aux2", [128, A2W], f32, isOutput=False)
    out_h = nc.declare_dram_parameter("out", [KSH, B], f32, isOutput=True)

    ctx = contextlib.ExitStack()

    def sb(name, shape, dt=f32):
        return ctx.enter_context(nc.sbuf_tensor(name, shape, dt))[:]

    def ps(name, shape, dt=f32):
        return ctx.enter_context(nc.psum_tensor(name, shape, dt))[:]

    with ctx:
        hrp_t = sb("hrp_t", [128, HW_])
        aux2_t = sb("aux2_t", [128, A2W])
        big_t = sb("big_t", [128, BW])
        ones_t = sb("ones_t", [128, 3, 512], u16)
        amt_t = sb("amt_t", [128, 3, 512], u16)
        w_t = sb("w_t", [128, 3, 512], u16)
        packed_hi_t = sb("packed_hi_t", [64, 3], u16)
        packed_t = sb("packed_t", [64, 3], u16)
        p33_t = sb("p33_t", [64, 33], u16)
        pres_t = sb("pres_t", [64, 33], bf16)
        ident_t = sb("ident_t", [64, 64], bf16)
        presT_t = sb("presT_t", [33, 64], bf16)
        biasg_t = sb("biasg_t", [128, NT])
        mask_t = sb("mask_t", [128, NT, KSH], bf16)
        ohlow_t = sb("ohlow_t", [33, IN], bf16)
        ohhigh_t = sb("ohhigh_t", [33, KSH], bf16)
        ohhsb_t = sb("ohhsb_t", [33, 2, KSH], bf16)
        xlT_t = sb("xlT_t", [128, NT, B], bf16)
        E_t = sb("E_t", [128, NT, KSH], bf16)
        sbso_t = sb("sbso_t", [KSH, 2, B])
        y1_t = sb("y1_t", [KSH, B])
        outT_t = sb("outT_t", [KSH, B])

        presT_ps = ps("presT_ps", [33, 64], bf16)
        plT_ps = ps("plT_ps", [128, NT, B])
        sb_ps = ps("sb_ps", [KSH, 2, B])
        Y_ps = ps("Y_ps", [KSH, B])

        hr_ap = hrp_t[:, H_HR : H_HR + 256].bitcast(i16)
        rlowpp_ap = aux2_t[:, A_RLPP : A_RLPP + NT]
        rhighbb_ap = aux2_t[:, A_RHBB : A_RHBB + KSH]
        shamt_ap = aux2_t[0:64, A_SHAMT : A_SHAMT + 17].bitcast(u16)[:, 0:33]
        ident_ap = aux2_t[0:64, A_IDENT : A_IDENT + 32].bitcast(bf16)
        viota_ap = aux2_t[0:33, A_VIOTA : A_VIOTA + 1]
        xT_ap = big_t[:, B_XT : B_XT + NT * B // 2].bitcast(bf16).rearrange(
            "p (t b) -> p t b", t=NT
        )
        dirT_ap = big_t[:, B_DIRT : B_DIRT + NT * KSH // 2].bitcast(bf16).rearrange(
            "p (t k) -> p t k", t=NT
        )
        v33 = big_t[0:33, B_V33 : B_V33 + VW // 2].bitcast(bf16)
        rlow33_ap = v33[:, V_RL : V_RL + IN]
        rhigh33_ap = v33[:, V_RH : V_RH + KSH]
        cs33_ap = v33[:, V_CS : V_CS + KSH]
        cb33_ap = v33[:, V_CB : V_CB + KSH]

        hr_sem = ctx.enter_context(nc.semaphore("hr_sem"))
        big_sem = ctx.enter_context(nc.semaphore("big_sem"))
        aux2_sem = ctx.enter_context(nc.semaphore("aux2_sem"))
        out_sem = ctx.enter_context(nc.semaphore("out_sem"))
        dve_sem = ctx.enter_context(nc.semaphore("dve_sem"))
        pe_sem = ctx.enter_context(nc.semaphore("pe_sem"))
        act_sem = ctx.enter_context(nc.semaphore("act_sem"))
        block = ctx.enter_context(nc.Block())

        @block.sync
        def _(sync):
            sync.dma_start(out=hrp_t, in_=hrp_h[:, :]).then_inc(hr_sem, 16)
            sync.dma_start(out=aux2_t, in_=aux2_h[:, :]).then_inc(aux2_sem, 16)
            sync.dma_start(out=big_t, in_=big_h[:, :]).then_inc(big_sem, 16)
            sync.wait_ge(dve_sem, 8)
            sync.dma_start(out=out_h[:, :], in_=outT_t).then_inc(out_sem, 16)
            sync.wait_ge(out_sem, 16)

        @block.vector
        def _(vector):
            def drain():
                if use_drains:
                    vector.drain()
            # ---- presence bit-pack on [128=(half,b), 512] ----
            nc.vector.memset(ones_t, 1)  # runs in the hr-DMA shadow
            vector.wait_ge(hr_sem, 16)  # hr landed
            # three uint16 words: w0 bits 0..14 <- values 0..14,
            # w1 bits 1..14 <- values 15..28, w2 bits 1..4 <- values 29..32
            # (each word's clamp-boundary bits are garbage and never read)
            nc.vector.tensor_scalar(
                out=amt_t[:, 0, :], in0=hr_ap, scalar1=15, scalar2=None,
                op0=Alu.min,
            )
            nc.vector.tensor_scalar(
                out=amt_t[:, 1, :], in0=hr_ap, scalar1=14, scalar2=0,
                op0=Alu.subtract, op1=Alu.max,
            )
            nc.vector.tensor_scalar(
                out=amt_t[:, 2, :], in0=hr_ap, scalar1=28, scalar2=0,
                op0=Alu.subtract, op1=Alu.max,
            )
            drain()  # DVE same-engine RAW edges need explicit drains
            nc.vector.tensor_scalar(
                out=amt_t[:, 1, :], in0=amt_t[:, 1, :], scalar1=15,
                scalar2=None, op0=Alu.min,
            )
            drain()
            nc.vector.tensor_tensor(
                out=w_t, in0=ones_t, in1=amt_t, op=Alu.logical_shift_left,
            )
            # Sign-mask bias for ScalarE: 0.5 - r_low  (ACT computes the S
            # masks in parallel with the tree below)
            vector.wait_ge(aux2_sem, 16)
            nc.vector.tensor_scalar(
                out=biasg_t, in0=rlowpp_ap, scalar1=-1.0, scalar2=0.5,
                op0=Alu.mult, op1=Alu.add,
            ).then_inc(dve_sem, 1)
            # dve=1: biasg ready (ACT can compute the 8 sign masks)

            # OR-tree along free axis: 512 -> 1
            s = 256
            while s >= 1:
                drain()
                nc.vector.tensor_tensor(
                    out=w_t[:, :, 0:s], in0=w_t[:, :, 0:s],
                    in1=w_t[:, :, s : 2 * s], op=Alu.bitwise_or,
                )
                s //= 2
            # combine the two row-halves (equal-base-partition rule: stage
            # the upper half through a copy first)
            drain()
            nc.vector.tensor_copy(out=packed_hi_t, in_=w_t[64:128, :, 0])
            drain()
            nc.vector.tensor_tensor(
                out=packed_t, in0=w_t[0:64, :, 0], in1=packed_hi_t,
                op=Alu.bitwise_or,
            )
            # ---- extract 33 presence bits (permuted column order: col c ->
            # value c+1 for c in 0..31, col 32 -> value 0; shamt matches) ----
            drain()
            nc.vector.tensor_tensor(
                out=p33_t[:, 0:14],
                in0=packed_t[:, 0:1].broadcast_to((64, 14)),
                in1=shamt_ap[:, 0:14], op=Alu.logical_shift_right,
            )
            nc.vector.tensor_tensor(
                out=p33_t[:, 14:28],
                in0=packed_t[:, 1:2].broadcast_to((64, 14)),
                in1=shamt_ap[:, 14:28], op=Alu.logical_shift_right,
            )
            nc.vector.tensor_tensor(
                out=p33_t[:, 28:32],
                in0=packed_t[:, 2:3].broadcast_to((64, 4)),
                in1=shamt_ap[:, 28:32], op=Alu.logical_shift_right,
            )
            nc.vector.tensor_copy(out=p33_t[:, 32:33], in_=packed_t[:, 0:1])
            drain()
            nc.vector.tensor_scalar(
                out=p33_t, in0=p33_t, scalar1=1, scalar2=None,
                op0=Alu.bitwise_and,
            )
            drain()
            nc.vector.tensor_copy(out=pres_t, in_=p33_t)
            nc.vector.tensor_copy(out=ident_t, in_=ident_ap).then_inc(dve_sem, 1)
            # dve=2: pres_t + ident_t ready (PE can transpose)

            # ---- one-hots over the (permuted) value axis: row r -> value
            # r+1 for r in 0..31, row 32 -> value 0 (viota matches) ----
            vector.wait_ge(big_sem, 16)  # big128 landed
            nc.vector.tensor_scalar(
                out=ohlow_t, in0=rlow33_ap, scalar1=viota_ap,
                scalar2=None, op0=Alu.is_equal,
            )
            vector.wait_ge(pe_sem, 1)  # presT_ps ready
            nc.vector.tensor_copy(out=presT_t, in_=presT_ps).then_inc(dve_sem, 1)
            # dve=3: ohlow + presT ready (PE can gather plT)

            nc.vector.tensor_scalar(
                out=ohhigh_t, in0=rhigh33_ap, scalar1=viota_ap,
                scalar2=None, op0=Alu.is_equal,
            )
            drain()
            nc.vector.tensor_mul(
                out=ohhsb_t,
                in0=ohhigh_t[:, None, :].broadcast_to((33, 2, KSH)),
                in1=v33[:, V_CS : V_CS + 2 * KSH].rearrange(
                    "v (c k) -> v c k", c=2
                ),
            ).then_inc(dve_sem, 1)
            # dve=4: scaled one-hots ready (PE can compute oms/phs)

            vector.wait_ge(pe_sem, 2)  # plT_ps ready
            nc.vector.tensor_mul(
                out=xlT_t, in0=xT_ap, in1=plT_ps
            ).then_inc(dve_sem, 1)
            # dve=5: xlT ready

            # ---- masked weights: E = mask * dirT (all bf16, 2x DVE mode),
            # in two halves so the PE main matmul chases ----
            vector.wait_ge(act_sem, 1)
            nc.vector.tensor_mul(
                out=E_t[:, 0 : NT // 2, :], in0=mask_t[:, 0 : NT // 2, :],
                in1=dirT_ap[:, 0 : NT // 2, :],
            ).then_inc(dve_sem, 1)
            # dve=6: E first half
            vector.wait_ge(act_sem, 2)
            nc.vector.tensor_mul(
                out=E_t[:, NT // 2 : NT, :], in0=mask_t[:, NT // 2 : NT, :],
                in1=dirT_ap[:, NT // 2 : NT, :],
            ).then_inc(dve_sem, 1)
            # dve=7: E second half

            vector.wait_ge(pe_sem, 3)  # oms/phs ready
            nc.vector.tensor_copy(out=sbso_t, in_=sb_ps)
            vector.wait_ge(pe_sem, 4)  # Y ready
            drain()
            nc.vector.tensor_mul(out=y1_t, in0=sbso_t[:, 0, :], in1=Y_ps)
            drain()
            nc.vector.tensor_add(
                out=outT_t, in0=y1_t, in1=sbso_t[:, 1, :]
            ).then_inc(dve_sem, 1)
            # dve=8: output ready in SBUF

        @block.scalar
        def _(scalar):
            scalar.wait_ge(aux2_sem, 16)  # rhighbb landed
            scalar.wait_ge(dve_sem, 1)    # biasg ready
            # mask[j,(t),k] = Relu(Sign(rhigh[k] - rlow[j] + 0.5)) in {0, 1}
            for t in range(NT):
                nc.scalar.activation(
                    out=mask_t[:, t, :], in_=rhighbb_ap, func=Act.Sign,
                    bias=biasg_t[:, t : t + 1], scale=1.0,
                )
            scalar.drain()
            for t in range(NT):
                ins = nc.scalar.activation(
                    out=mask_t[:, t, :], in_=mask_t[:, t, :], func=Act.Relu,
                    bias=0.0, scale=1.0,
                )
                if t == NT // 2 - 1:
                    ins.then_inc(act_sem, 1)
            ins.then_inc(act_sem, 1)

        @block.tensor
        def _(tensor):
            tensor.wait_ge(dve_sem, 2)
            nc.tensor.transpose(presT_ps, pres_t, ident_t).then_inc(pe_sem, 1)
            # pe=1: presT_ps ready
            tensor.wait_ge(dve_sem, 3)
            # plT[j, b] = present[b, r_low[j]] * (r_low[j] != 0)
            for t in range(NT):
                ins = nc.tensor.matmul(
                    plT_ps[:, t, :],
                    ohlow_t[0:32, t * 128 : (t + 1) * 128],
                    presT_t[0:32, :],
                )
            ins.then_inc(pe_sem, 1)
            # pe=2: plT ready
            tensor.wait_ge(dve_sem, 4)
            # phs[k,b] = cscale[k]*ph;  oms[k,b] = cbias[k]*om
            nc.tensor.matmul(sb_ps[:, 0, :], ohhsb_t[0:32, 0, :], presT_t[0:32, :])
            nc.tensor.matmul(
                sb_ps[:, 1, :], ohhsb_t[0:33, 1, :], presT_t[0:33, :]
            ).then_inc(pe_sem, 1)
            # pe=3: oms/phs ready
            tensor.wait_ge(dve_sem, 6)  # xlT (5) + E first half (6)
            for t in range(NT // 2):
                nc.tensor.matmul(
                    Y_ps, E_t[:, t, :], xlT_t[:, t, :],
                    start=(t == 0), stop=False,
                )
            tensor.wait_ge(dve_sem, 7)  # E second half
            for t in range(NT // 2, NT):
                ins = nc.tensor.matmul(
                    Y_ps, E_t[:, t, :], xlT_t[:, t, :],
                    start=False, stop=(t == NT - 1),
                )
            ins.then_inc(pe_sem, 1)
            # pe=4: Y ready

    return nc


def _host_tables():
    """Input-independent constant tables (shift amounts, identity)."""
    import ml_dtypes

    shamt = np.zeros((64, 34), np.uint16)
    shamt[:, 0:14] = np.arange(1, 15)[None, :]    # values 1..14 in word0
    shamt[:, 14:28] = np.arange(1, 15)[None, :]   # values 15..28 in word1
    shamt[:, 28:32] = np.arange(1, 5)[None, :]    # values 29..32 in word2
    shamt[:, 32] = 0                              # value 0 in word0
    ident = np.eye(64, dtype=ml_dtypes.bfloat16)
    viota = np.empty((33, 1), np.float32)
    viota[0:32, 0] = np.arange(1, 33)
    viota[32, 0] = 0.0
    return shamt, ident, viota


def _prep_in_maps(inputs):
    """Host-side sharding: layout / dtype transforms only, no arithmetic."""
    import ml_dtypes

    bf = ml_dtypes.bfloat16
    x = np.ascontiguousarray(np.asarray(inputs["x"], dtype=np.float32))
    hr = np.ascontiguousarray(np.asarray(inputs["hidden_rank"], dtype=np.int32))
    r_low = np.asarray(inputs["r_low"], dtype=np.int32)
    r_high = np.asarray(inputs["r_high"], dtype=np.int32)
    direction = np.asarray(inputs["direction"], dtype=np.float32)
    cscale_b = np.asarray(inputs["cscale_b"], dtype=np.float32)
    cbias_b = np.asarray(inputs["cbias_b"], dtype=np.float32)

    # partition p = h*64 + b, free = s: hr2[h*64+b, s] = hr[b, h*512+s]
    hr2 = hr.reshape(B, 2, 512).transpose(1, 0, 2).reshape(128, 512)
    # xT3[p, t, b] = x[b, t*128+p]
    xT3 = x.T.reshape(NT, 128, B).transpose(1, 0, 2)
    rlowf = r_low.astype(np.float32)
    rhighf = r_high.astype(np.float32)
    shamt, ident, viota = _host_tables()

    hrp = np.zeros((128, HW_), np.float32)
    hrp[:, H_HR : H_HR + 256] = hr2.astype(np.int16).view(np.float32)

    aux2 = np.zeros((128, A2W), np.float32)
    aux2[:, A_RLPP : A_RLPP + NT] = rlowf.reshape(NT, 128).T
    aux2[0:64, A_SHAMT : A_SHAMT + 17] = shamt.view(np.float32)
    aux2[0:64, A_IDENT : A_IDENT + 32] = ident.view(np.float32)
    aux2[0:33, A_VIOTA : A_VIOTA + 1] = viota

    in_maps = []
    for c in range(NCORES):
        sl = slice(c * KSH, (c + 1) * KSH)
        rh = rhighf[sl]
        aux2c = aux2.copy()
        aux2c[:, A_RHBB : A_RHBB + KSH] = rh[None, :]
        big = np.zeros((128, BW), np.float32)
        big[:, B_XT : B_XT + NT * B // 2] = (
            xT3.reshape(128, -1).astype(bf).view(np.float32)
        )
        big[:, B_DIRT : B_DIRT + NT * KSH // 2] = (
            direction[sl, :].T.reshape(NT, 128, KSH).transpose(1, 0, 2)
            .reshape(128, -1).astype(bf).view(np.float32)
        )
        vals = np.zeros((33, VW), bf)
        vals[:, V_RL : V_RL + IN] = rlowf[None, :].astype(bf)
        vals[:, V_RH : V_RH + KSH] = rh[None, :].astype(bf)
        vals[:, V_CS : V_CS + KSH] = cscale_b[sl][None, :].astype(bf)
        vals[:, V_CB : V_CB + KSH] = cbias_b[sl][None, :].astype(bf)
        big[0:33, B_V33 : B_V33 + VW // 2] = vals.view(np.float32)
        in_maps.append({"hrp": hrp, "big128": big, "aux2": aux2c})
    return in_maps


def _run(inputs, trace=False, **kw):
    from concourse.bass_utils import run_bass_kernel_spmd

    if "nc" not in _cached:
        _cached["nc"] = _build_nc()
    nc = _cached["nc"]
    in_maps = _prep_in_maps(inputs)
    res = run_bass_kernel_spmd(
        nc, in_maps, core_ids=list(range(NCORES)), trace=trace, **kw
    )
    out = np.concatenate([np.asarray(r["out"]).T for r in res.results], axis=1)
    return out.astype(np.float32), res


def kernel(**inputs):
    out, _ = _run(inputs, trace=False)
    return out
